# revision 24
# baseline (speedup 1.0000x reference)
"""Trainium2 Bass kernel for nn_AttentionLayer (attention pooling).

Reference math (per batch row b):
    u   = tanh(x[b] @ W + b_vec)        # [T, M]
    s   = u @ us                        # [T]
    a   = softmax(s) * mask / sum       # [T]  (mask is all ones per spec)
    out = a @ x[b]                      # [D]

Strategy: data-parallel over batch, B=32 rows -> 4 rows per NeuronCore on
8 cores.  Per core the kernel is HBM-DMA-bound (~85-95us to stream x at
the per-core share of stack bandwidth), so the emission is built around
keeping that stream and the PE continuously busy:
  - x streams in as 1MB half-quarter DMAs on the sync HWDGE queue with a
    deep lookahead; constants and y outputs use the ACT HWDGE queue and
    single-descriptor layouts so they never stall behind the x stream;
  - fp32->bf16 casts per quarter on DVE; PE transposes x 128x128 blocks
    via identity matmuls (~56ns/tile back-to-back);
  - u^T = tanh(W^T x^T + bias) accumulates in PSUM, tanh fused on ScalarE
    (W arrives host-rearranged in lhsT layout, b/us arrive as rows and
    are transposed on the PE);
  - scores via matmul(lhsT=u^T chunk, rhs=us); exp on ScalarE (no max
    subtraction needed: |s| <= ~5, exact softmax up to fp32 rounding);
  - out = (e^T x) / sum(e): four PE column-groups compute four d-slices
    concurrently into one PSUM bank; a ones-column appended to x_bf makes
    column group 3 also accumulate sum(e), so normalization is just a
    reciprocal and four scaled copies.
The PE warm-up keeps the HAM clock gate at K=8/8 through the first real
work, with filler matmuls interleaved into quarter 0's transpose-only
stretch so the activity window never lapses.
"""
import numpy as np

import concourse.bacc as bacc
import concourse.mybir as mybir
from concourse.tile import TileContext
from concourse.masks import make_identity
from concourse.bass_utils import run_bass_kernel_spmd

F32 = mybir.dt.float32
BF16 = mybir.dt.bfloat16

B, T, D, M = 32, 2048, 1024, 128
NCORES = 8
B_SH = B // NCORES   # 4 batch rows per core
P = 128
NT = T // P          # 16 t-tiles per row
NCD = D // P         # 8 d-chunks
QT = 4               # t-tiles per quarter-row
NQ = NT // QT        # 4 quarters per row
NH = B_SH * NT // 2  # 32 half-quarter DMAs
DP = D + 2           # x_bf inner size: D cols + ones column + pad
WARMUP = 80
WARMUP_FILL = 24
LOOKAHEAD = 6


def _build_nc():
    nc = bacc.Bacc("TRN2", target_bir_lowering=False, debug=False,
                   num_devices=NCORES)
    x = nc.declare_dram_parameter("x", [B_SH, T, D], F32, isOutput=False)
    # W arrives host-rearranged to lhsT layout: W_r[p, c, m] = W[128c+p, m];
    # b and us arrive as rows so each is a single DMA descriptor
    W = nc.declare_dram_parameter("W", [P, NCD, M], F32, isOutput=False)
    b = nc.declare_dram_parameter("b", [1, M], F32, isOutput=False)
    us = nc.declare_dram_parameter("us", [1, M], F32, isOutput=False)
    y = nc.declare_dram_parameter("y", [B_SH, D], F32, isOutput=True)

    with TileContext(nc) as tc:
        with (
            tc.tile_pool(name="singles", bufs=1) as singles,
            tc.tile_pool(name="stage", bufs=4) as stage,
            tc.tile_pool(name="xb", bufs=2) as xb_pool,
            tc.tile_pool(name="xt", bufs=3) as xt_pool,
            tc.tile_pool(name="u", bufs=2) as u_pool,
            tc.tile_pool(name="et", bufs=4) as et_pool,
            tc.tile_pool(name="outs", bufs=2) as out_pool,
            tc.tile_pool(name="tp_ps", bufs=3, space="PSUM") as tp_psum,
            tc.tile_pool(name="u_ps", bufs=2, space="PSUM") as u_psum,
            tc.tile_pool(name="s_ps", bufs=1, space="PSUM") as s_psum,
            tc.tile_pool(name="o_ps", bufs=2, space="PSUM") as o_psum,
        ):
            # constants on the ACT HWDGE queue FIRST so they beat the x flood
            w_f32 = singles.tile([P, NCD, M], F32)
            nc.scalar.dma_start(out=w_f32, in_=W[:, :, :])
            b_row = singles.tile([1, M], F32)
            nc.scalar.dma_start(out=b_row, in_=b[:, :])
            us_row = singles.tile([1, M], F32)
            nc.scalar.dma_start(out=us_row, in_=us[:, :])

            # x streams on the sync HWDGE queue: quarter tiles, each filled
            # by two 1MB half DMAs so the stream stays fine-grained
            stage_tiles = {}
            emitted = [0]

            def emit_stage():
                h = emitted[0]
                idx, half = divmod(h, 2)
                r, q = divmod(idx, NQ)
                if half == 0:
                    st_new = stage.tile([P, QT, D], F32, tag="stage",
                                        name=f"stage_{idx}")
                    stage_tiles[idx] = st_new
                st = stage_tiles[idx]
                nc.sync.dma_start(
                    out=st[:, 2 * half:2 * half + 2, :],
                    in_=x[r].rearrange("(n p) d -> p n d", p=P)[
                        :, q * QT + 2 * half:q * QT + 2 * half + 2, :],
                )
                emitted[0] += 1

            emit_stage()  # first x tile right behind the constants

            # small on-chip constants (DVE + gpsimd)
            wones = singles.tile([P, P], BF16)
            nc.vector.memset(wones, 1.0)
            one_f32 = singles.tile([1, 1], F32)
            nc.vector.memset(one_f32, 1.0)

            ident = singles.tile([P, P], BF16)
            make_identity(nc, ident)

            w_bf = singles.tile([P, NCD, M], BF16)
            nc.vector.tensor_copy(out=w_bf, in_=w_f32)

            while emitted[0] < LOOKAHEAD:
                emit_stage()

            # PE warm-up: lift HAM to K=8/8 while the first DMAs stream and
            # hand off to the first transposes without an idle window
            warm = u_psum.tile([P, QT * P], F32, tag="up")
            for i in range(WARMUP):
                nc.tensor.matmul(warm[:, :P], wones, wones, start=True, stop=True)

            # transpose b/us rows to per-partition layout on the PE
            bc_ps = s_psum.tile([P, 2], F32, tag="s")
            nc.tensor.matmul(bc_ps[:, 0:1], b_row, one_f32, start=True, stop=True)
            nc.tensor.matmul(bc_ps[:, 1:2], us_row, one_f32, start=True, stop=True)
            b_sb = singles.tile([P, 1], F32)
            nc.vector.tensor_copy(out=b_sb, in_=bc_ps[:, 0:1])
            us_bf = singles.tile([P, 1], BF16)
            nc.vector.tensor_copy(out=us_bf, in_=bc_ps[:, 1:2])

            # software pipeline over all quarters; c1/c2 op lists for
            # quarter idx drain interleaved with transposes of idx+1/idx+2
            rowstate = {}
            compute1 = {}
            compute2 = {}

            def drain(ops, k):
                for _ in range(k):
                    if ops:
                        ops.pop(0)()

            def warm_fill(n=WARMUP_FILL):
                ops = []
                for i in range(n):
                    def f():
                        nc.tensor.matmul(warm[:, :P], wones, wones,
                                         start=True, stop=True)
                    ops.append(f)
                return ops

            compute1[-1] = warm_fill()

            for idx in range(B_SH * NQ):
                r, q = divmod(idx, NQ)
                if q == 0:
                    rowstate[r] = dict(
                        x_bf=xb_pool.tile([P, NT, DP], BF16, tag="xb", name=f"x_bf_{r}"),
                        u_sb=u_pool.tile([P, T], BF16, tag="u", name=f"u_sb_{r}"),
                        op=o_psum.tile([P, 257], F32, tag="o", name=f"op_{r}"),
                        o_sb=out_pool.tile([1, D], F32, tag="o_sb", name=f"o_sb_{r}"),
                    )
                    # ones column for the sum(e) accumulator in column group 3
                    nc.vector.memset(rowstate[r]["x_bf"][:, :, D:D + 1], 1.0)
                rs_ = rowstate[r]
                x_bf, u_sb = rs_["x_bf"], rs_["u_sb"]

                # cast for this quarter (DVE), with the DMA stream kept
                # LOOKAHEAD halves ahead
                while emitted[0] < min(idx * 2 + LOOKAHEAD, NH):
                    emit_stage()
                st = stage_tiles.pop(idx)
                nc.gpsimd.tensor_copy(
                    out=x_bf[:, q * QT:(q + 1) * QT, 0:D], in_=st)

                c1 = compute1.pop(idx - 1, [])
                c2 = compute2.pop(idx - 2, [])

                # transposes for this quarter, interleaved with draining the
                # previous quarters' matmul work
                xt = xt_pool.tile([P, QT, NCD, P], BF16, tag="xt")
                for j in range(QT):
                    t_idx = q * QT + j
                    tp = tp_psum.tile([P, NCD * P], BF16, tag="tp")
                    for c in range(NCD):
                        nc.tensor.transpose(
                            tp[:, c * P:(c + 1) * P],
                            x_bf[:, t_idx, c * P:(c + 1) * P],
                            ident,
                        )
                        if c == 3:
                            drain(c1, 2)
                    dst = xt[:, j, :, :]
                    src = tp.rearrange("p (c t) -> p c t", c=NCD)
                    if j == 0:
                        nc.scalar.copy(out=dst, in_=src)
                    else:
                        nc.vector.tensor_copy(out=dst, in_=src)
                    drain(c1, 2)
                    drain(c2, 1)
                drain(c1, len(c1))
                drain(c2, len(c2))

                def make_c1(r=r, q=q, xt=xt, u_sb=u_sb, rs_=rs_):
                    ops = []
                    up = u_psum.tile([P, QT * P], F32, tag="up")

                    def mk_p1(c):
                        def f():
                            nc.tensor.matmul(
                                up, w_bf[:, c, :], xt[:, :, c, :],
                                start=(c == 0), stop=(c == NCD - 1),
                            )
                        return f
                    for c in range(NCD):
                        ops.append(mk_p1(c))

                    def tanh_op():
                        nc.scalar.activation(
                            out=u_sb[:, q * QT * P:(q + 1) * QT * P], in_=up,
                            func=mybir.ActivationFunctionType.Tanh,
                            bias=b_sb, scale=1.0,
                        )
                    ops.append(tanh_op)

                    sp = s_psum.tile([P, QT], F32, tag="s")

                    def mk_st(j):
                        def f():
                            t_idx = q * QT + j
                            nc.tensor.matmul(
                                sp[:, j:j + 1],
                                u_sb[:, t_idx * P:(t_idx + 1) * P],
                                us_bf, start=True, stop=True,
                            )
                        return f
                    for j in range(QT):
                        ops.append(mk_st(j))

                    etq = et_pool.tile([P, QT], BF16, tag="et")
                    rs_[f"et{q}"] = etq

                    def exp_op():
                        nc.scalar.activation(
                            out=etq, in_=sp,
                            func=mybir.ActivationFunctionType.Exp,
                        )
                    ops.append(exp_op)
                    return ops

                compute1[idx] = make_c1()

                def make_c2(r=r, q=q, x_bf=x_bf, rs_=rs_):
                    # each list entry emits one t-tile's four column-group
                    # matmuls back-to-back so they co-run on the PE; group 3
                    # carries the ones column to accumulate sum(e)
                    ops = []

                    def mk_p2(j):
                        def f():
                            t_idx = q * QT + j
                            op_t = rs_["op"]
                            for g in range(4):
                                n = 257 if g == 3 else 256
                                kwargs = {}
                                if g > 0:
                                    kwargs["tile_position"] = (0, 32 * g)
                                nc.tensor.matmul(
                                    op_t[32 * g:32 * g + 1, 0:n],
                                    rs_[f"et{q}"][:, j:j + 1],
                                    x_bf[:, t_idx, 256 * g:256 * g + n],
                                    start=(q == 0 and j == 0),
                                    stop=(q == NQ - 1 and j == QT - 1),
                                    **kwargs,
                                )
                        return f
                    for j in range(QT):
                        ops.append(mk_p2(j))

                    if q == NQ - 1:
                        def finish():
                            op_t = rs_["op"]
                            inv = out_pool.tile([1, 1], F32, tag="inv")
                            nc.vector.reciprocal(out=inv, in_=op_t[96:97, 256:257])
                            o_sb = rs_["o_sb"]
                            for g in range(4):
                                nc.vector.tensor_scalar_mul(
                                    o_sb[:, 256 * g:256 * g + 256],
                                    op_t[32 * g:32 * g + 1, 0:256], inv)
                            nc.sync.dma_start(out=y[r:r + 1, :], in_=o_sb)
                        ops.append(finish)
                    return ops

                compute2[idx] = make_c2()

            for idx in sorted(set(compute1) | set(compute2)):
                for f in compute1.pop(idx, []):
                    f()
                for f in compute2.pop(idx, []):
                    f()

    nc.compile()
    return nc


_NC_CACHE = []


def _numpy_reference(x, W, b, us, mask):
    m = mask.astype(x.dtype)
    u = np.tanh(np.einsum('btd,dm->btm', x, W) + b)
    utu = np.einsum('btm,mo->bto', u, us)[..., 0]
    e = np.exp(utu - utu.max(axis=-1, keepdims=True))
    e = m * e
    a = e / e.sum(axis=-1, keepdims=True)
    return np.einsum('bt,btd->bd', a, x).astype(np.float32)


def make_in_maps(x, W, b, us):
    """Per-core input dicts; W goes in host-rearranged lhsT layout and
    b/us as single-descriptor rows."""
    x = np.ascontiguousarray(np.asarray(x, dtype=np.float32))
    W = np.ascontiguousarray(np.asarray(W, dtype=np.float32))
    b = np.ascontiguousarray(np.asarray(b, dtype=np.float32))
    us = np.ascontiguousarray(np.asarray(us, dtype=np.float32))
    W_r = np.ascontiguousarray(W.reshape(NCD, P, M).transpose(1, 0, 2))
    b_r = np.ascontiguousarray(b.reshape(1, M))
    us_r = np.ascontiguousarray(us.reshape(M, 1).T)
    return [{
        "x": np.ascontiguousarray(x[i * B_SH:(i + 1) * B_SH]),
        "W": W_r, "b": b_r, "us": us_r,
    } for i in range(NCORES)]


def kernel(x, W, b, us, mask):
    x = np.ascontiguousarray(np.asarray(x, dtype=np.float32))
    W = np.ascontiguousarray(np.asarray(W, dtype=np.float32))
    b = np.ascontiguousarray(np.asarray(b, dtype=np.float32))
    us = np.ascontiguousarray(np.asarray(us, dtype=np.float32))
    mask = np.asarray(mask)

    if not bool(mask.all()):
        # spec guarantees an all-ones mask; fall back to exact numpy
        # reference if that ever changes
        return _numpy_reference(x, W, b, us, mask)

    if not _NC_CACHE:
        _NC_CACHE.append(_build_nc())
    nc = _NC_CACHE[0]

    in_maps = make_in_maps(x, W, b, us)
    res = run_bass_kernel_spmd(nc, in_maps, core_ids=list(range(NCORES)),
                               trace=False)
    return np.concatenate([res.results[i]["y"] for i in range(NCORES)], axis=0)


# revision 26
# speedup vs baseline: 1.8199x; 1.8199x over previous
"""Trainium2 Bass kernel for nn_AttentionLayer (attention pooling).

Reference math (per batch row b):
    u   = tanh(x[b] @ W + b_vec)        # [T, M]
    s   = u @ us                        # [T]
    a   = softmax(s) * mask / sum       # [T]  (mask is all ones per spec)
    out = a @ x[b]                      # [D]

Strategy: data-parallel over batch, B=32 rows -> 4 rows per NeuronCore on
8 cores.  Per core the kernel is HBM-DMA-bound (~85-95us to stream x at
the per-core share of stack bandwidth), so the emission is built around
keeping that stream and the PE continuously busy:
  - x streams in as 1MB half-quarter DMAs on the sync HWDGE queue with a
    deep lookahead; constants and y outputs use the ACT HWDGE queue and
    single-descriptor layouts so they never stall behind the x stream;
  - fp32->bf16 casts per quarter on DVE; PE transposes x 128x128 blocks
    via identity matmuls (~56ns/tile back-to-back);
  - u^T = tanh(W^T x^T + bias) accumulates in PSUM, tanh fused on ScalarE
    (W arrives host-rearranged in lhsT layout, b/us arrive as rows and
    are transposed on the PE);
  - scores via matmul(lhsT=u^T chunk, rhs=us); exp on ScalarE (no max
    subtraction needed: |s| <= ~5, exact softmax up to fp32 rounding);
  - out = (e^T x) / sum(e): four PE column-groups compute four d-slices
    concurrently into one PSUM bank; a ones-column appended to x_bf makes
    column group 3 also accumulate sum(e), so normalization is just a
    reciprocal and four scaled copies.
The PE warm-up keeps the HAM clock gate at K=8/8 through the first real
work, with filler matmuls interleaved into quarter 0's transpose-only
stretch so the activity window never lapses.
"""
import numpy as np

import concourse.bacc as bacc
import concourse.mybir as mybir
from concourse.tile import TileContext
from concourse.masks import make_identity
from concourse.bass_utils import run_bass_kernel_spmd

F32 = mybir.dt.float32
BF16 = mybir.dt.bfloat16

B, T, D, M = 32, 2048, 1024, 128
NCORES = 8
B_SH = B // NCORES   # 4 batch rows per core
P = 128
NT = T // P          # 16 t-tiles per row
NCD = D // P         # 8 d-chunks
QT = 4               # t-tiles per quarter-row
NQ = NT // QT        # 4 quarters per row
NH = B_SH * NT // 2  # 32 half-quarter DMAs
DP = D + 2           # x_bf inner size: D cols + ones column + pad
WARMUP = 80
WARMUP_FILL = 24
LOOKAHEAD = 6


def _build_nc():
    nc = bacc.Bacc("TRN2", target_bir_lowering=False, debug=False,
                   num_devices=NCORES)
    x = nc.declare_dram_parameter("x", [B_SH, T, D], F32, isOutput=False)
    # W arrives host-rearranged to lhsT layout: W_r[p, c, m] = W[128c+p, m];
    # b and us arrive as rows so each is a single DMA descriptor
    W = nc.declare_dram_parameter("W", [P, NCD, M], F32, isOutput=False)
    b = nc.declare_dram_parameter("b", [1, M], F32, isOutput=False)
    us = nc.declare_dram_parameter("us", [1, M], F32, isOutput=False)
    y = nc.declare_dram_parameter("y", [B_SH, D], F32, isOutput=True)

    with TileContext(nc) as tc:
        with (
            tc.tile_pool(name="singles", bufs=1) as singles,
            tc.tile_pool(name="stage", bufs=4) as stage,
            tc.tile_pool(name="xb", bufs=2) as xb_pool,
            tc.tile_pool(name="xt", bufs=3) as xt_pool,
            tc.tile_pool(name="u", bufs=2) as u_pool,
            tc.tile_pool(name="et", bufs=4) as et_pool,
            tc.tile_pool(name="outs", bufs=2) as out_pool,
            tc.tile_pool(name="tp_ps", bufs=3, space="PSUM") as tp_psum,
            tc.tile_pool(name="u_ps", bufs=2, space="PSUM") as u_psum,
            tc.tile_pool(name="s_ps", bufs=1, space="PSUM") as s_psum,
            tc.tile_pool(name="o_ps", bufs=2, space="PSUM") as o_psum,
        ):
            # constants on the ACT HWDGE queue FIRST so they beat the x flood
            w_f32 = singles.tile([P, NCD, M], F32)
            nc.scalar.dma_start(out=w_f32, in_=W[:, :, :])
            b_row = singles.tile([1, M], F32)
            nc.scalar.dma_start(out=b_row, in_=b[:, :])
            us_row = singles.tile([1, M], F32)
            nc.scalar.dma_start(out=us_row, in_=us[:, :])

            # x streams on the sync HWDGE queue: quarter tiles, each filled
            # by two 1MB half DMAs so the stream stays fine-grained
            stage_tiles = {}
            emitted = [0]

            def emit_stage():
                h = emitted[0]
                idx, half = divmod(h, 2)
                r, q = divmod(idx, NQ)
                if half == 0:
                    st_new = stage.tile([P, QT, D], F32, tag="stage",
                                        name=f"stage_{idx}")
                    stage_tiles[idx] = st_new
                st = stage_tiles[idx]
                nc.sync.dma_start(
                    out=st[:, 2 * half:2 * half + 2, :],
                    in_=x[r].rearrange("(n p) d -> p n d", p=P)[
                        :, q * QT + 2 * half:q * QT + 2 * half + 2, :],
                )
                emitted[0] += 1

            emit_stage()  # first x tile right behind the constants

            # small on-chip constants (DVE + gpsimd)
            wones = singles.tile([P, P], BF16)
            nc.vector.memset(wones, 1.0)
            one_f32 = singles.tile([1, 1], F32)
            nc.vector.memset(one_f32, 1.0)

            ident = singles.tile([P, P], BF16)
            make_identity(nc, ident)

            w_bf = singles.tile([P, NCD, M], BF16)
            nc.vector.tensor_copy(out=w_bf, in_=w_f32)

            while emitted[0] < LOOKAHEAD:
                emit_stage()

            # PE warm-up: lift HAM to K=8/8 while the first DMAs stream and
            # hand off to the first transposes without an idle window
            warm = u_psum.tile([P, QT * P], F32, tag="up")
            for i in range(WARMUP):
                nc.tensor.matmul(warm[:, :P], wones, wones, start=True, stop=True)

            # transpose b/us rows to per-partition layout on the PE
            bc_ps = s_psum.tile([P, 2], F32, tag="s")
            nc.tensor.matmul(bc_ps[:, 0:1], b_row, one_f32, start=True, stop=True)
            nc.tensor.matmul(bc_ps[:, 1:2], us_row, one_f32, start=True, stop=True)
            b_sb = singles.tile([P, 1], F32)
            nc.vector.tensor_copy(out=b_sb, in_=bc_ps[:, 0:1])
            us_bf = singles.tile([P, 1], BF16)
            nc.vector.tensor_copy(out=us_bf, in_=bc_ps[:, 1:2])

            # software pipeline over all quarters; c1/c2 op lists for
            # quarter idx drain interleaved with transposes of idx+1/idx+2
            rowstate = {}
            compute1 = {}
            compute2 = {}

            def drain(ops, k):
                for _ in range(k):
                    if ops:
                        ops.pop(0)()

            def warm_fill(n=WARMUP_FILL):
                ops = []
                for i in range(n):
                    def f():
                        nc.tensor.matmul(warm[:, :P], wones, wones,
                                         start=True, stop=True)
                    ops.append(f)
                return ops

            compute1[-1] = warm_fill()

            def ensure_row(r):
                if r not in rowstate:
                    rowstate[r] = dict(
                        x_bf=xb_pool.tile([P, NT, DP], BF16, tag="xb", name=f"x_bf_{r}"),
                        u_sb=u_pool.tile([P, T], BF16, tag="u", name=f"u_sb_{r}"),
                        op=o_psum.tile([P, 257], F32, tag="o", name=f"op_{r}"),
                        o_sb=out_pool.tile([1, D], F32, tag="o_sb", name=f"o_sb_{r}"),
                    )
                    # ones column for the sum(e) accumulator in column group 3
                    nc.vector.memset(rowstate[r]["x_bf"][:, :, D:D + 1], 1.0)
                return rowstate[r]

            def emit_cast(idx):
                r, q = divmod(idx, NQ)
                rs_ = ensure_row(r)
                st = stage_tiles.pop(idx)
                nc.vector.tensor_copy(
                    out=rs_["x_bf"][:, q * QT:(q + 1) * QT, 0:D], in_=st)

            for idx in range(B_SH * NQ):
                r, q = divmod(idx, NQ)
                while emitted[0] < min(idx * 2 + LOOKAHEAD, NH):
                    emit_stage()
                if idx == 0:
                    emit_cast(0)
                rs_ = rowstate[r]
                x_bf, u_sb = rs_["x_bf"], rs_["u_sb"]

                c1 = compute1.pop(idx - 1, [])
                c2 = compute2.pop(idx - 2, [])

                # transposes for this quarter, interleaved with draining the
                # previous quarters' matmul work
                xt = xt_pool.tile([P, QT, NCD, P], BF16, tag="xt")
                for j in range(QT):
                    t_idx = q * QT + j
                    tp = tp_psum.tile([P, NCD * P], BF16, tag="tp")
                    for c in range(NCD):
                        nc.tensor.transpose(
                            tp[:, c * P:(c + 1) * P],
                            x_bf[:, t_idx, c * P:(c + 1) * P],
                            ident,
                        )
                        if c == 3:
                            drain(c1, 2)
                    dst = xt[:, j, :, :]
                    src = tp.rearrange("p (c t) -> p c t", c=NCD)
                    if j % 2 == 0:
                        nc.scalar.copy(out=dst, in_=src)
                    else:
                        nc.vector.tensor_copy(out=dst, in_=src)
                    drain(c1, 2)
                    drain(c2, 1)
                drain(c1, len(c1))
                drain(c2, len(c2))
                # prefetch-cast the next quarter so the copies above sit
                # ahead of it in the DVE FIFO
                if idx + 1 < B_SH * NQ:
                    emit_cast(idx + 1)

                def make_c1(r=r, q=q, xt=xt, u_sb=u_sb, rs_=rs_):
                    ops = []
                    up = u_psum.tile([P, QT * P], F32, tag="up")

                    def mk_p1(c):
                        def f():
                            nc.tensor.matmul(
                                up, w_bf[:, c, :], xt[:, :, c, :],
                                start=(c == 0), stop=(c == NCD - 1),
                            )
                        return f
                    for c in range(NCD):
                        ops.append(mk_p1(c))

                    def tanh_op():
                        nc.scalar.activation(
                            out=u_sb[:, q * QT * P:(q + 1) * QT * P], in_=up,
                            func=mybir.ActivationFunctionType.Tanh,
                            bias=b_sb, scale=1.0,
                        )
                    ops.append(tanh_op)

                    sp = s_psum.tile([P, QT], F32, tag="s")

                    def mk_st(j):
                        def f():
                            t_idx = q * QT + j
                            nc.tensor.matmul(
                                sp[:, j:j + 1],
                                u_sb[:, t_idx * P:(t_idx + 1) * P],
                                us_bf, start=True, stop=True,
                            )
                        return f
                    for j in range(QT):
                        ops.append(mk_st(j))

                    etq = et_pool.tile([P, QT], BF16, tag="et")
                    rs_[f"et{q}"] = etq

                    def exp_op():
                        nc.scalar.activation(
                            out=etq, in_=sp,
                            func=mybir.ActivationFunctionType.Exp,
                        )
                    ops.append(exp_op)
                    return ops

                compute1[idx] = make_c1()

                def make_c2(r=r, q=q, x_bf=x_bf, rs_=rs_):
                    # each list entry emits one t-tile's four column-group
                    # matmuls back-to-back so they co-run on the PE; group 3
                    # carries the ones column to accumulate sum(e)
                    ops = []

                    def mk_p2(j):
                        def f():
                            t_idx = q * QT + j
                            op_t = rs_["op"]
                            for g in range(4):
                                n = 257 if g == 3 else 256
                                kwargs = {}
                                if g > 0:
                                    kwargs["tile_position"] = (0, 32 * g)
                                nc.tensor.matmul(
                                    op_t[32 * g:32 * g + 1, 0:n],
                                    rs_[f"et{q}"][:, j:j + 1],
                                    x_bf[:, t_idx, 256 * g:256 * g + n],
                                    start=(q == 0 and j == 0),
                                    stop=(q == NQ - 1 and j == QT - 1),
                                    **kwargs,
                                )
                        return f
                    for j in range(QT):
                        ops.append(mk_p2(j))

                    if q == NQ - 1:
                        def finish():
                            op_t = rs_["op"]
                            inv = out_pool.tile([1, 1], F32, tag="inv")
                            nc.vector.reciprocal(out=inv, in_=op_t[96:97, 256:257])
                            o_sb = rs_["o_sb"]
                            for g in range(4):
                                nc.vector.tensor_scalar_mul(
                                    o_sb[:, 256 * g:256 * g + 256],
                                    op_t[32 * g:32 * g + 1, 0:256], inv)
                            nc.sync.dma_start(out=y[r:r + 1, :], in_=o_sb)
                        ops.append(finish)
                    return ops

                compute2[idx] = make_c2()

            for idx in sorted(set(compute1) | set(compute2)):
                for f in compute1.pop(idx, []):
                    f()
                for f in compute2.pop(idx, []):
                    f()

    nc.compile()
    return nc


_NC_CACHE = []


def _numpy_reference(x, W, b, us, mask):
    m = mask.astype(x.dtype)
    u = np.tanh(np.einsum('btd,dm->btm', x, W) + b)
    utu = np.einsum('btm,mo->bto', u, us)[..., 0]
    e = np.exp(utu - utu.max(axis=-1, keepdims=True))
    e = m * e
    a = e / e.sum(axis=-1, keepdims=True)
    return np.einsum('bt,btd->bd', a, x).astype(np.float32)


def make_in_maps(x, W, b, us):
    """Per-core input dicts; W goes in host-rearranged lhsT layout and
    b/us as single-descriptor rows."""
    x = np.ascontiguousarray(np.asarray(x, dtype=np.float32))
    W = np.ascontiguousarray(np.asarray(W, dtype=np.float32))
    b = np.ascontiguousarray(np.asarray(b, dtype=np.float32))
    us = np.ascontiguousarray(np.asarray(us, dtype=np.float32))
    W_r = np.ascontiguousarray(W.reshape(NCD, P, M).transpose(1, 0, 2))
    b_r = np.ascontiguousarray(b.reshape(1, M))
    us_r = np.ascontiguousarray(us.reshape(M, 1).T)
    return [{
        "x": np.ascontiguousarray(x[i * B_SH:(i + 1) * B_SH]),
        "W": W_r, "b": b_r, "us": us_r,
    } for i in range(NCORES)]


def kernel(x, W, b, us, mask):
    x = np.ascontiguousarray(np.asarray(x, dtype=np.float32))
    W = np.ascontiguousarray(np.asarray(W, dtype=np.float32))
    b = np.ascontiguousarray(np.asarray(b, dtype=np.float32))
    us = np.ascontiguousarray(np.asarray(us, dtype=np.float32))
    mask = np.asarray(mask)

    if not bool(mask.all()):
        # spec guarantees an all-ones mask; fall back to exact numpy
        # reference if that ever changes
        return _numpy_reference(x, W, b, us, mask)

    if not _NC_CACHE:
        _NC_CACHE.append(_build_nc())
    nc = _NC_CACHE[0]

    in_maps = make_in_maps(x, W, b, us)
    res = run_bass_kernel_spmd(nc, in_maps, core_ids=list(range(NCORES)),
                               trace=False)
    return np.concatenate([res.results[i]["y"] for i in range(NCORES)], axis=0)
